# revision 20
# baseline (speedup 1.0000x reference)
"""Trainium2 Bass kernel for nn_ConvG (3-level GCN + TopK pooling + readout).

Data-parallel over 8 NeuronCores (16 graphs each). Host converts the edge
list to dense per-graph adjacency W = A + I (bf16), pre-transposes x, and
additionally precomputes for level 1 (where the keep-mask is all-ones):
  - dinv1 = (col-sums of W)^-1/2 as ready-made scale columns, and
  - MADJ = W @ diag(dinv1^2) @ W, which fuses the two propagation hops of
    level 1 into a single dense matmul pass.

Key structural points vs. a naive pipeline:
  - The TopK pool scale cs = kd*kv*tanh(s) is NOT materialized into the
    feature map. The next dense layer consumes raw HRAW with a folded
    per-node drain scale c1 = kd_next*cs (valid since kd_next >= 0:
    kd*relu(cs*z) == relu(kd*cs*z)), so no pooled copy of h exists.
  - Readouts come from two fused DVE tensor_tensor_reduce ops per
    (graph, feature-half): product cs*HRAW with max-accum (seeded at 0 --
    dropped nodes contribute exactly 0, and HRAW >= 0 up to tiny relu
    zeros, so seeding at 0 matches the reference max to ~3e-4 absolute)
    and with sum-accum (exact: dropped nodes contribute cs = 0).
  - All transcendentals use one activation-table set
    (natural_log_exp_and_others): dinv = exp(-0.5*ln(deg)), tanh computed
    as 1 - 2/(exp(2s)+1), and the final log_softmax's Exp/Ln.
  - hop2 / fused-hop psum pairs drain 512 wide; score columns multiply
    kd directly from PSUM; level boundaries keep per-graph slice deps so
    Tile can pipeline graphs across stages.
"""
import numpy as np

G = 16            # graphs per core
N = 256           # nodes per graph
F_IN = 128
H1 = 256
H2 = 128
C = 10
NCORES = 8
B = G * NCORES    # 128 graphs
KS = [205, 164, 132]
DROPS = [51, 41, 32]
BIG = 1e30
MINV = -1e30

_CACHE = {}


def _build(with_bias, split_psum=False, use_ttr=False, use_expln=True):
    import concourse.bacc as bacc
    import concourse.mybir as mybir
    import concourse.tile as tile
    from concourse.masks import make_identity

    f32 = mybir.dt.float32
    bf16 = mybir.dt.bfloat16
    AF = mybir.ActivationFunctionType
    OP = mybir.AluOpType
    AX = mybir.AxisListType

    nc = bacc.Bacc("TRN2", target_bir_lowering=False, debug=False)

    GN = G * N  # 4096

    xt_d = nc.dram_tensor("xt", [F_IN, GN], bf16, kind="ExternalInput")
    adj_d = nc.dram_tensor("adj", [G, N, N], bf16, kind="ExternalInput")
    madj_d = nc.dram_tensor("madj", [G, N, N], bf16, kind="ExternalInput")
    kd1_d = nc.dram_tensor("kd1", [128, 2 * G], f32, kind="ExternalInput")
    esel_d = nc.dram_tensor("esel", [G, G * 128], bf16, kind="ExternalInput")
    w12_d = nc.dram_tensor("w12", [F_IN, H1], bf16, kind="ExternalInput")
    w22_d = nc.dram_tensor("w22", [H1, H1], bf16, kind="ExternalInput")
    w32_d = nc.dram_tensor("w32", [H1, H1], bf16, kind="ExternalInput")
    w1_d = nc.dram_tensor("w1", [2 * H1, H1], bf16, kind="ExternalInput")
    w2_d = nc.dram_tensor("w2", [H1, H2], bf16, kind="ExternalInput")
    w3_d = nc.dram_tensor("w3", [H2, C], bf16, kind="ExternalInput")
    pwc_d = nc.dram_tensor("pwc", [128, 6], bf16, kind="ExternalInput")
    b12_d = nc.dram_tensor("b12", [1, H1], bf16, kind="ExternalInput")
    b22_d = nc.dram_tensor("b22", [1, H1], bf16, kind="ExternalInput")
    b32_d = nc.dram_tensor("b32", [1, H1], bf16, kind="ExternalInput")
    b1_d = nc.dram_tensor("b1", [1, H1], bf16, kind="ExternalInput")
    b2_d = nc.dram_tensor("b2", [1, H2], bf16, kind="ExternalInput")
    b3_d = nc.dram_tensor("b3", [1, C], bf16, kind="ExternalInput")
    out_d = nc.dram_tensor("out", [G, C], f32, kind="ExternalOutput")

    with tile.TileContext(nc) as tc:
        import contextlib
        with contextlib.ExitStack() as ctx:
            big = ctx.enter_context(tc.tile_pool(name="big", bufs=1))
            sm = ctx.enter_context(tc.tile_pool(name="sm", bufs=1))
            hmp = ctx.enter_context(tc.tile_pool(name="hmp", bufs=4))
            pmm = ctx.enter_context(tc.tile_pool(name="pmm", bufs=5, space="PSUM"))
            pt = ctx.enter_context(tc.tile_pool(name="pt", bufs=1, space="PSUM"))
            pv = ctx.enter_context(tc.tile_pool(name="pv", bufs=2, space="PSUM"))

            ADJ = big.tile([128, 2 * GN], bf16, tag="adj")
            MADJ = big.tile([128, 2 * GN], bf16, tag="madj")
            XT = big.tile([128, GN], bf16, tag="xt")
            U = big.tile([128, 2 * GN], bf16, tag="u")
            U2 = big.tile([128, 2 * GN], bf16, tag="u2")
            # HRAW: (g, ft)-major bf16: chunk (g, ft) at (2g+ft)*N
            HRAW = big.tile([128, 2 * GN], bf16, tag="hraw")
            # CSSB2: cs broadcast, doubled to match HRAW's (g, ft) pairs
            CSSB2 = big.tile([128, 2 * GN], bf16, tag="cssb2")
            if with_bias:
                HMF = big.tile([128, 2 * GN], bf16, tag="hmf")

            ESEL = sm.tile([G, G * 128], bf16, tag="esel")
            W12S = sm.tile([128, H1], bf16, tag="w12s")
            W22S = sm.tile([128, 2 * H1], bf16, tag="w22s")
            W32S = sm.tile([128, 2 * H1], bf16, tag="w32s")
            W1S = sm.tile([128, 4 * H1], bf16, tag="w1s")
            W2S = sm.tile([128, 2 * H2], bf16, tag="w2s")
            W3S = sm.tile([128, C], bf16, tag="w3s")
            PWC = sm.tile([128, 6], bf16, tag="pwc")
            B12R = sm.tile([1, H1], bf16, tag="b12r")
            B22R = sm.tile([1, H1], bf16, tag="b22r")
            B32R = sm.tile([1, H1], bf16, tag="b32r")
            B1R = sm.tile([1, H1], bf16, tag="b1r")
            B2R = sm.tile([1, H2], bf16, tag="b2r")
            B3R = sm.tile([1, C], bf16, tag="b3r")
            BLV = {0: B12R, 1: B22R, 2: B32R}

            IDT = sm.tile([128, 128], f32, tag="idt")
            ONEB = sm.tile([1, 128], bf16, tag="oneb")
            EPSB = sm.tile([128, 1], f32, tag="epsb")

            # column tiles [128, 2G]: col = mt*G + g
            LNC = sm.tile([128, 2 * G], f32, tag="lnc")
            DICB = sm.tile([128, 2 * G], f32, tag="dicb")
            KVCA = sm.tile([128, 2 * G], f32, tag="kvca")
            KDCA = sm.tile([128, 2 * G], f32, tag="kdca")
            KD2CA = sm.tile([128, 2 * G], f32, tag="kd2ca")
            C1C = sm.tile([128, 2 * G], f32, tag="c1c")
            SCOL = sm.tile([128, 2 * G], f32, tag="scol")
            ECOL = sm.tile([128, 2 * G], f32, tag="ecol")
            DCOL = sm.tile([128, 2 * G], f32, tag="dcol")
            RCOL = sm.tile([128, 2 * G], f32, tag="rcol")
            THC = sm.tile([128, 2 * G], f32, tag="thc")
            CSC = sm.tile([128, 2 * G], f32, tag="csc")
            KVT = [sm.tile([128, G], bf16, tag=f"kvt{i}", name=f"KVT{i}")
                   for i in range(2)]

            # row tiles [16, 256] (one graph per partition)
            KV = sm.tile([16, N], f32, tag="kv")
            KVN = sm.tile([16, N], f32, tag="kvn")
            S = sm.tile([16, N], f32, tag="s")
            AM16 = sm.tile([16, N], f32, tag="am16")
            DS = sm.tile([16, N], f32, tag="ds")
            WRK = sm.tile([16, N], f32, tag="wrk")
            T1 = sm.tile([16, N], f32, tag="t1")
            CSB16 = sm.tile([16, N], bf16, tag="csb16")
            TK8 = sm.tile([16, 8], f32, tag="tk8")

            # readout accumulators: col = 2g + ft
            RDTX = [sm.tile([128, 2 * G], f32, tag=f"rdtx{l}",
                            name=f"RDTX{l}") for l in range(3)]
            RDTS = [sm.tile([128, 2 * G], f32, tag=f"rdts{l}",
                            name=f"RDTS{l}") for l in range(3)]
            ZACC = sm.tile([128, 64], f32, tag="zacc")
            SCRC = sm.tile([128, G], f32, tag="scrc")

            Z1 = sm.tile([16, H1], f32, tag="z1")
            Z1T = sm.tile([128, 2 * G], bf16, tag="z1t")
            Z2 = sm.tile([16, H2], f32, tag="z2")
            Z2T = sm.tile([128, G], bf16, tag="z2t")
            M16 = sm.tile([16, 1], f32, tag="m16")
            NM16 = sm.tile([16, 1], f32, tag="nm16")
            ES = sm.tile([16, C], f32, tag="es")
            SE = sm.tile([16, 1], f32, tag="se")
            LSE = sm.tile([16, 1], f32, tag="lse")
            OUTS = sm.tile([16, C], f32, tag="outs")

            def usl(g, t):  # U/U2 column slice for (graph, node-half)
                o = (g * 2 + t) * N
                return slice(o, o + N)

            def asl(g, st):  # ADJ/MADJ block (graph, src-half): [s128, d256]
                o = (g * 2 + st) * N
                return slice(o, o + N)

            def col(g, mt):
                return slice(mt * G + g, mt * G + g + 1)

            def hch(g, ft):  # HRAW chunk slice for (graph, feat-half)
                o = (2 * g + ft) * N
                return slice(o, o + N)

            def csl(g):
                return slice(g * N, (g + 1) * N)

            # ---- consts + input DMAs
            make_identity(nc, IDT[:])
            nc.gpsimd.memset(ONEB[:], 1.0)
            nc.gpsimd.memset(EPSB[:], 1e-12)
            nc.gpsimd.memset(KV[:], 1.0)

            nc.sync.dma_start(XT[:], xt_d[:])
            nc.sync.dma_start(W12S[:], w12_d[:])
            nc.sync.dma_start(KDCA[:], kd1_d[:])
            nc.sync.dma_start(PWC[:], pwc_d[:])
            # MADJ in 4-graph chunks so level-1 fused prop can start early
            for c4 in range(4):
                gs = slice(c4 * 4, c4 * 4 + 4)
                nc.sync.dma_start(
                    MADJ[:, c4 * 2048:(c4 + 1) * 2048].rearrange(
                        "p (g t d) -> p g t d", g=4, t=2),
                    madj_d[gs].rearrange("g (t p) d -> p g t d", p=128))
            for c4 in range(4):
                gs = slice(c4 * 4, c4 * 4 + 4)
                nc.sync.dma_start(
                    ADJ[:, c4 * 2048:(c4 + 1) * 2048].rearrange(
                        "p (g t d) -> p g t d", g=4, t=2),
                    adj_d[gs].rearrange("g (t p) d -> p g t d", p=128))
            nc.sync.dma_start(ESEL[:], esel_d[:])
            nc.sync.dma_start(W22S[:].rearrange("p (t n) -> p t n", n=H1),
                              w22_d[:].rearrange("(t p) n -> p t n", p=128))
            nc.sync.dma_start(W32S[:].rearrange("p (t n) -> p t n", n=H1),
                              w32_d[:].rearrange("(t p) n -> p t n", p=128))
            nc.sync.dma_start(W1S[:].rearrange("p (t n) -> p t n", n=H1),
                              w1_d[:].rearrange("(t p) n -> p t n", p=128))
            nc.sync.dma_start(W2S[:].rearrange("p (t n) -> p t n", n=H2),
                              w2_d[:].rearrange("(t p) n -> p t n", p=128))
            nc.sync.dma_start(W3S[:], w3_d[:])
            for dst, src in ((B12R, b12_d), (B22R, b22_d), (B32R, b32_d),
                             (B1R, b1_d), (B2R, b2_d), (B3R, b3_d)):
                nc.sync.dma_start(dst[:], src[:])

            def deg_c1():
                """deg cols = W^T kv -> dinv via exp(-ln/2) -> kd/kd2/c1."""
                pdg = pv.tile([128, 2 * G], f32, tag="pcol")
                for g in range(G):
                    for dh in range(2):
                        for st in range(2):
                            ao = (g * 2 + st) * N + dh * 128
                            nc.tensor.matmul(pdg[:, col(g, dh)],
                                             ADJ[:, ao:ao + 128],
                                             KVT[st][:, g:g + 1],
                                             start=(st == 0), stop=(st == 1))
                if use_expln:
                    nc.scalar.activation(LNC[:], pdg[:], AF.Ln,
                                         bias=EPSB[:, 0:1])
                    nc.scalar.activation(DICB[:], LNC[:], AF.Exp, scale=-0.5)
                else:
                    nc.scalar.activation(LNC[:], pdg[:], AF.Sqrt,
                                         bias=EPSB[:, 0:1])
                    nc.vector.reciprocal(DICB[:], LNC[:])
                nc.vector.tensor_mul(KDCA[:], DICB[:], KVCA[:])
                nc.vector.tensor_mul(KD2CA[:], KDCA[:], DICB[:])
                nc.vector.tensor_mul(C1C[:], KDCA[:], CSC[:])

            def dense(lvl):
                """U = scale o relu(h @ W), node-major; stationary = h."""
                if lvl == 0:
                    WS, kts = W12S, 1
                else:
                    WS = {1: W22S, 2: W32S}[lvl]
                    kts = 2
                for g in range(G):
                    for mt in range(2):
                        ps = pmm.tile([128, H1], f32, tag="ps")
                        for kt in range(kts):
                            if lvl == 0:
                                lhs = XT[:, g * N + mt * 128:
                                         g * N + mt * 128 + 128]
                            elif with_bias:
                                lhs = HMF[:, (2 * g + kt) * N + mt * 128:
                                          (2 * g + kt) * N + mt * 128 + 128]
                            else:
                                lhs = HRAW[:, (2 * g + kt) * N + mt * 128:
                                           (2 * g + kt) * N + mt * 128 + 128]
                            nc.tensor.matmul(ps[:], lhs,
                                             WS[:, kt * H1:(kt + 1) * H1],
                                             start=(kt == 0),
                                             stop=(not with_bias and
                                                   kt == kts - 1))
                        if with_bias:
                            nc.tensor.matmul(ps[:], ONEB[0:1, :], BLV[lvl][:],
                                             start=False, stop=True)
                        sc = KDCA if (lvl == 0 or with_bias) else C1C
                        dst = U[:, usl(g, mt)]
                        if g % 2 == 0:
                            nc.scalar.activation(dst, ps[:], AF.Relu,
                                                 scale=sc[:, col(g, mt)])
                        else:
                            nc.vector.tensor_scalar(dst, ps[:],
                                                    sc[:, col(g, mt)], 0.0,
                                                    op0=OP.mult, op1=OP.max)

            def hop_out(g, AD, UIN):
                """Feature-major 2-MM-group hop out of AD with stationary
                chunks of UIN; drains raw into HRAW pair slice."""
                if split_psum:
                    for ft in range(2):
                        ps = pmm.tile([128, H1], f32, tag="ps")
                        for eh in range(2):
                            uo = (g * 2 + eh) * N + ft * 128
                            nc.tensor.matmul(ps[:], UIN[:, uo:uo + 128],
                                             AD[:, asl(g, eh)],
                                             start=(eh == 0), stop=(eh == 1))
                        dst = HRAW[:, hch(g, ft)]
                        if (2 * g + ft) % 2 == 0:
                            nc.scalar.copy(dst, ps[:])
                        else:
                            nc.vector.tensor_copy(dst, ps[:])
                else:
                    P = pmm.tile([128, 2 * H1], f32, tag="ps")
                    for ft in range(2):
                        for eh in range(2):
                            uo = (g * 2 + eh) * N + ft * 128
                            nc.tensor.matmul(P[:, ft * H1:(ft + 1) * H1],
                                             UIN[:, uo:uo + 128],
                                             AD[:, asl(g, eh)],
                                             start=(eh == 0), stop=(eh == 1))
                    dst = HRAW[:, 2 * g * N:2 * g * N + 2 * N]
                    if g % 2 == 0:
                        nc.scalar.copy(dst, P[:])
                    else:
                        nc.vector.tensor_copy(dst, P[:])

            def fused_prop1():
                # p2 = (W D^2 W)^T-contracted in one pass: stationary = U
                for g in range(G):
                    hop_out(g, MADJ, U)

            def prop23():
                # hop1: u2 = kd2 o (W^T u), node-major
                for g in range(G):
                    for dh in range(2):
                        ps = pmm.tile([128, H1], f32, tag="ps")
                        for st in range(2):
                            ao = (g * 2 + st) * N + dh * 128
                            nc.tensor.matmul(ps[:], ADJ[:, ao:ao + 128],
                                             U[:, usl(g, st)],
                                             start=(st == 0), stop=(st == 1))
                        dst = U2[:, usl(g, dh)]
                        if g % 2 == 0:
                            nc.scalar.activation(dst, ps[:], AF.Copy,
                                                 scale=KD2CA[:, col(g, dh)])
                        else:
                            nc.vector.tensor_scalar_mul(dst, ps[:],
                                                        KD2CA[:, col(g, dh)])
                # hop2: p2 = W^T u2, FEATURE-major; raw drain to HRAW
                for g in range(G):
                    hop_out(g, ADJ, U2)

            def trow(dst_row, src_col_ap, mt, out_bf=False):
                """[128, G] column-tile slice -> row-tile [16, 128] block."""
                pp = pt.tile([128, 128], f32, tag="pst")
                nc.tensor.transpose(pp[0:16, :], src_col_ap, IDT[:])
                eng = nc.scalar.copy if out_bf else nc.vector.tensor_copy
                eng(dst_row[0:16, mt * 128:(mt + 1) * 128], pp[0:16, :])

            def score(lvl):
                """score cols s = kd o (pw . p2); tanh via exp identity."""
                psc = pv.tile([128, 2 * G], f32, tag="pcol")
                for g in range(G):
                    for mt in range(2):
                        for ft in range(2):
                            ho = (2 * g + ft) * N + mt * 128
                            nc.tensor.matmul(
                                psc[:, col(g, mt)],
                                HRAW[:, ho:ho + 128],
                                PWC[:, lvl * 2 + ft:lvl * 2 + ft + 1],
                                start=(ft == 0), stop=(ft == 1))
                nc.vector.tensor_mul(SCOL[:], psc[:], KDCA[:])
                nc.scalar.activation(THC[:], SCOL[:], AF.Tanh)
                for mt in range(2):
                    trow(S, SCOL[:, mt * G:(mt + 1) * G], mt)

            def topk_pool(lvl):
                d = DROPS[lvl]
                # mask inactive scores; drop-side top-k
                nc.vector.tensor_scalar(AM16[:], KV[:], 1.0, BIG,
                                        op0=OP.subtract, op1=OP.mult)
                nc.vector.tensor_sub(DS[:], AM16[:], S[:])
                cur = DS
                for it in range((d + 7) // 8):
                    nc.vector.max(TK8[:], cur[:])
                    rem = d - it * 8
                    if rem < 8:
                        nc.vector.memset(TK8[:, rem:8], MINV)
                    nc.vector.match_replace(WRK[:], TK8[:], cur[:], MINV)
                    cur = WRK
                # kv_new: 1 where WRK is a kept score (-s), 0 elsewhere
                nc.vector.tensor_scalar(T1[:], WRK[:], 1e-29, 1.0,
                                        op0=OP.mult, op1=OP.add)
                nc.vector.tensor_scalar(KVN[:], T1[:], 0.0, 1.0,
                                        op0=OP.max, op1=OP.min)
                nc.vector.tensor_copy(KV[:], KVN[:])
                # kv columns (fp32 + bf16) for next level's deg
                for mt in range(2):
                    pp = pt.tile([128, 128], f32, tag="pst")
                    nc.tensor.transpose(pp[:, 0:16],
                                        KVN[0:16, mt * 128:(mt + 1) * 128],
                                        IDT[0:16, 0:16])
                    nc.scalar.copy(KVCA[:, mt * G:(mt + 1) * G], pp[:, 0:16])
                    nc.vector.tensor_copy(KVT[mt][:], pp[:, 0:16])
                # cs = kd*kv_new*tanh(s) as columns -> bf16 rows
                nc.vector.tensor_mul(CSC[:], KDCA[:], KVCA[:])
                nc.vector.tensor_mul(CSC[:], CSC[:], THC[:])
                for mt in range(2):
                    trow(CSB16, CSC[:, mt * G:(mt + 1) * G], mt, out_bf=True)
                # selector broadcast rows -> psum pair -> SBUF bf16 (doubled)
                for g in range(G):
                    cb = pmm.tile([128, 2 * N], f32, tag="ps")
                    for half in range(2):
                        nc.tensor.matmul(cb[:, half * N:(half + 1) * N],
                                         ESEL[:, g * 128:(g + 1) * 128],
                                         CSB16[:], start=True, stop=True)
                    dst = CSSB2[:, 2 * g * N:2 * g * N + 2 * N]
                    if g % 2 == 0:
                        nc.scalar.copy(dst, cb[:])
                    else:
                        nc.vector.tensor_copy(dst, cb[:])

            def readout(lvl):
                """GpSimd pair products; DVE max pair-reduce + 4x sum-accum.
                Max seeds at 0 implicitly: dropped nodes contribute cs=0."""
                for g in range(G):
                    if with_bias:
                        hm2 = HMF[:, 2 * g * N:2 * g * N + 2 * N]
                    else:
                        hmt = hmp.tile([128, 2 * N], bf16, tag="hm")
                        hm2 = hmt[:]
                    nc.gpsimd.tensor_mul(hm2, HRAW[:, 2 * g * N:
                                                   2 * g * N + 2 * N],
                                         CSSB2[:, 2 * g * N:
                                               2 * g * N + 2 * N])
                    for ft in range(2):
                        rc = 2 * g + ft
                        sdum = hmp.tile([128, N], bf16, tag="sdum")
                        nc.vector.tensor_scalar(
                            sdum[:], hm2[:, ft * N:(ft + 1) * N], 1.0, None,
                            op0=OP.mult, op1=OP.add,
                            accum_out=RDTS[lvl][:, rc:rc + 1])
                        sdum2 = hmp.tile([128, N], bf16, tag="sdum")
                        nc.vector.tensor_scalar(
                            sdum2[:], hm2[:, ft * N:(ft + 1) * N], 1.0, None,
                            op0=OP.mult, op1=OP.max,
                            accum_out=RDTX[lvl][:, rc:rc + 1])

            # ---- the network
            dense(0)
            fused_prop1()
            score(0)
            topk_pool(0)
            readout(0)
            for lvl in range(1, 3):
                deg_c1()
                dense(lvl)
                prop23()
                score(lvl)
                topk_pool(lvl)
                readout(lvl)

            # ---- combine readouts: z = sum_lvl [max | mean/k]
            def ftview(t, ft):
                # [128, 2G] (g, ft)-major -> [128, 1, G] slice for this ft
                return t[:].rearrange("p (g f) -> p f g", f=2)[:, ft:ft + 1, :]

            for kind in range(2):
                RD = RDTX if kind == 0 else RDTS
                for ft in range(2):
                    cg = (kind * 2 + ft) * G
                    dst = ZACC[:, cg:cg + G].rearrange("p (f g) -> p f g",
                                                       f=1)
                    v0, v1, v2 = (ftview(RD[l], ft) for l in range(3))
                    if kind == 0:
                        nc.vector.tensor_add(dst, v0, v1)
                        nc.vector.tensor_add(dst, dst, v2)
                    else:
                        nc.vector.tensor_scalar_mul(dst, v0, 1.0 / KS[0])
                        for l2, vv in ((1, v1), (2, v2)):
                            s3 = SCRC[:].rearrange("p (f g) -> p f g", f=1)
                            nc.vector.tensor_scalar_mul(s3, vv, 1.0 / KS[l2])
                            nc.vector.tensor_add(dst, dst, s3)

            # ---- final MLP + log_softmax
            ZB = sm.tile([128, 64], bf16, tag="zb")
            nc.vector.tensor_copy(ZB[:], ZACC[:])
            ps1 = pv.tile([16, H1], f32, tag="pcol")
            for kt in range(4):
                nc.tensor.matmul(ps1[0:16, :], ZB[:, kt * 16:(kt + 1) * 16],
                                 W1S[:, kt * H1:(kt + 1) * H1],
                                 start=(kt == 0), stop=False)
            nc.tensor.matmul(ps1[0:16, :], ONEB[0:1, 0:16], B1R[:],
                             start=False, stop=True)
            nc.scalar.activation(Z1[:], ps1[0:16, :], AF.Relu)
            for kt in range(2):
                pp = pt.tile([128, 128], f32, tag="pst")
                nc.tensor.transpose(pp[:, 0:16],
                                    Z1[0:16, kt * 128:(kt + 1) * 128],
                                    IDT[0:16, 0:16])
                nc.scalar.copy(Z1T[:, kt * G:(kt + 1) * G], pp[:, 0:16])
            ps2 = pv.tile([16, H2], f32, tag="pcol")
            for kt in range(2):
                nc.tensor.matmul(ps2[0:16, :], Z1T[:, kt * G:(kt + 1) * G],
                                 W2S[:, kt * H2:(kt + 1) * H2],
                                 start=(kt == 0), stop=False)
            nc.tensor.matmul(ps2[0:16, :], ONEB[0:1, 0:16], B2R[:],
                             start=False, stop=True)
            nc.scalar.activation(Z2[:], ps2[0:16, :], AF.Relu)
            pp = pt.tile([128, 128], f32, tag="pst")
            nc.tensor.transpose(pp[:, 0:16], Z2[0:16, :], IDT[0:16, 0:16])
            nc.scalar.copy(Z2T[:], pp[:, 0:16])
            ps3 = pv.tile([16, C], f32, tag="pcol")
            nc.tensor.matmul(ps3[0:16, :], Z2T[:], W3S[:], start=True,
                             stop=False)
            nc.tensor.matmul(ps3[0:16, :], ONEB[0:1, 0:16], B3R[:],
                             start=False, stop=True)
            nc.vector.tensor_reduce(M16[:], ps3[0:16, :], axis=AX.X, op=OP.max)
            nc.vector.tensor_scalar_mul(NM16[:], M16[:], -1.0)
            nc.scalar.activation(ES[:], ps3[0:16, :], AF.Exp,
                                 bias=NM16[0:16, 0:1], scale=1.0)
            nc.vector.tensor_reduce(SE[:], ES[:], axis=AX.X, op=OP.add)
            nc.scalar.activation(LSE[:], SE[:], AF.Ln)
            nc.vector.tensor_scalar(OUTS[:], ps3[0:16, :], M16[0:16, 0:1],
                                    LSE[0:16, 0:1], op0=OP.subtract,
                                    op1=OP.subtract)
            nc.sync.dma_start(out_d[:], OUTS[:])

    nc.compile()
    return nc


def _get_nc(with_bias, **kw):
    key = f"nc{int(with_bias)}{sorted(kw.items())}"
    if key not in _CACHE:
        _CACHE[key] = _build(with_bias, **kw)
    return _CACHE[key]


def _host_prep(inputs):
    import ml_dtypes
    bfd = ml_dtypes.bfloat16
    x = np.asarray(inputs["x"], np.float32)
    edges = np.asarray(inputs["edges"], np.int32)
    src = edges[..., 0].astype(np.int64)
    dst = edges[..., 1].astype(np.int64)
    gidx = np.arange(B, dtype=np.int64)[:, None]
    flat = (gidx * N * N + src * N + dst).ravel()
    A = np.bincount(flat, minlength=B * N * N).astype(np.float32)
    A = A.reshape(B, N, N)
    A += np.eye(N, dtype=np.float32)[None]

    # level-1 norms (keep-mask all ones) + fused 2-hop matrix
    degk = A.sum(axis=1)                       # [B, N]: col sums of W
    dinv1 = degk ** -0.5
    MADJ = np.matmul(A * (dinv1 ** 2)[:, None, :], A)  # (W D^2 W)[e, d]

    Ab = A.astype(bfd)
    Mb = MADJ.astype(bfd)
    xt = np.ascontiguousarray(
        x.reshape(NCORES, G * N, F_IN).transpose(0, 2, 1)).astype(bfd)

    # kd1 column tiles per core: [128, 2G], col = mt*G + g
    kd1 = np.zeros((NCORES, 128, 2 * G), np.float32)
    for c in range(NCORES):
        dv = dinv1[c * G:(c + 1) * G]          # [G, N]
        for mt in range(2):
            kd1[c, :, mt * G:(mt + 1) * G] = dv[:, mt * 128:(mt + 1) * 128].T

    esel = np.zeros((G, G * 128), np.float32)
    for g in range(G):
        esel[g, g * 128:(g + 1) * 128] = 1.0

    shared = {"esel": esel.astype(bfd)}
    for name, key in (("w12", "W12"), ("w22", "W22"), ("w32", "W32"),
                      ("w1", "W1"), ("w2", "W2"), ("w3", "W3")):
        shared[name] = np.ascontiguousarray(
            np.asarray(inputs[key], np.float32).astype(bfd))
    for name, key, n in (("b12", "b12", H1), ("b22", "b22", H1),
                         ("b32", "b32", H1), ("b1", "b1", H1),
                         ("b2", "b2", H2), ("b3", "b3", C)):
        shared[name] = np.asarray(inputs[key], np.float32).reshape(1, n) \
            .astype(bfd)
    pwc = np.zeros((128, 6), np.float32)
    for i, key in enumerate(("pw1", "pw2", "pw3")):
        pw = np.asarray(inputs[key], np.float32)
        pwn = pw / np.linalg.norm(pw)
        pwc[:, 2 * i] = pwn[:128]
        pwc[:, 2 * i + 1] = pwn[128:]
    shared["pwc"] = pwc.astype(bfd)

    with_bias = any(np.any(np.asarray(inputs[k])) for k in
                    ("b12", "b22", "b32"))
    in_maps = []
    for c in range(NCORES):
        m = dict(shared)
        m["xt"] = np.ascontiguousarray(xt[c])
        m["adj"] = np.ascontiguousarray(Ab[c * G:(c + 1) * G])
        m["madj"] = np.ascontiguousarray(Mb[c * G:(c + 1) * G])
        m["kd1"] = np.ascontiguousarray(kd1[c])
        in_maps.append(m)
    return in_maps, with_bias


def kernel(**inputs):
    from concourse.bass_utils import run_bass_kernel_spmd
    in_maps, with_bias = _host_prep(inputs)
    nc = _get_nc(with_bias)
    r = run_bass_kernel_spmd(nc, in_maps, core_ids=list(range(NCORES)))
    out = np.concatenate([r.results[c]["out"] for c in range(NCORES)], axis=0)
    return out.astype(np.float32)


def run_traced(inputs):
    """Like kernel() but with NTFF tracing; returns (out, BassKernelResults)."""
    import sys
    import types
    if "antenv.axon_hooks" not in sys.modules:
        hooks = types.ModuleType("antenv.axon_hooks")
        hooks._hook = None
        hooks.set_axon_ntff_profile_hook = lambda h: setattr(hooks, "_hook", h)
        hooks.get_axon_ntff_profile_hook = lambda: hooks._hook
        sys.modules["antenv.axon_hooks"] = hooks
        from trn_agent_boot.trn_boot import _ntff_profile_via_ctypes
        hooks.set_axon_ntff_profile_hook(
            _ntff_profile_via_ctypes("/opt/axon/libaxon_pjrt.so"))
    from concourse.bass_utils import run_bass_kernel_spmd
    in_maps, with_bias = _host_prep(inputs)
    nc = _get_nc(with_bias)
    r = run_bass_kernel_spmd(nc, in_maps, core_ids=list(range(NCORES)),
                             trace=True)
    out = np.concatenate([r.results[c]["out"] for c in range(NCORES)], axis=0)
    return out.astype(np.float32), r


# revision 21
# speedup vs baseline: 1.0309x; 1.0309x over previous
"""Trainium2 Bass kernel for nn_ConvG (3-level GCN + TopK pooling + readout).

Data-parallel over 8 NeuronCores (16 graphs each). Host converts the edge
list to dense per-graph adjacency W = A + I (bf16), pre-transposes x, and
additionally precomputes for level 1 (where the keep-mask is all-ones):
  - dinv1 = (col-sums of W)^-1/2 as ready-made scale columns, and
  - MADJ = W @ diag(dinv1^2) @ W, which fuses the two propagation hops of
    level 1 into a single dense matmul pass.

Key structural points vs. a naive pipeline:
  - The TopK pool scale cs = kd*kv*tanh(s) is NOT materialized into the
    feature map. The next dense layer consumes raw HRAW with a folded
    per-node drain scale c1 = kd_next*cs (valid since kd_next >= 0:
    kd*relu(cs*z) == relu(kd*cs*z)), so no pooled copy of h exists.
  - Readouts come from two fused DVE tensor_tensor_reduce ops per
    (graph, feature-half): product cs*HRAW with max-accum (seeded at 0 --
    dropped nodes contribute exactly 0, and HRAW >= 0 up to tiny relu
    zeros, so seeding at 0 matches the reference max to ~3e-4 absolute)
    and with sum-accum (exact: dropped nodes contribute cs = 0).
  - All transcendentals use one activation-table set
    (natural_log_exp_and_others): dinv = exp(-0.5*ln(deg)), tanh computed
    as 1 - 2/(exp(2s)+1), and the final log_softmax's Exp/Ln.
  - hop2 / fused-hop psum pairs drain 512 wide; score columns multiply
    kd directly from PSUM; level boundaries keep per-graph slice deps so
    Tile can pipeline graphs across stages.
"""
import numpy as np

G = 16            # graphs per core
N = 256           # nodes per graph
F_IN = 128
H1 = 256
H2 = 128
C = 10
NCORES = 8
B = G * NCORES    # 128 graphs
KS = [205, 164, 132]
DROPS = [51, 41, 32]
BIG = 1e30
MINV = -1e30

_CACHE = {}


def _build(with_bias, split_psum=False, use_ttr=False, use_expln=True):
    import concourse.bacc as bacc
    import concourse.mybir as mybir
    import concourse.tile as tile
    from concourse.masks import make_identity

    f32 = mybir.dt.float32
    bf16 = mybir.dt.bfloat16
    AF = mybir.ActivationFunctionType
    OP = mybir.AluOpType
    AX = mybir.AxisListType

    nc = bacc.Bacc("TRN2", target_bir_lowering=False, debug=False)

    GN = G * N  # 4096

    xt_d = nc.dram_tensor("xt", [F_IN, GN], bf16, kind="ExternalInput")
    adj_d = nc.dram_tensor("adj", [G, N, N], bf16, kind="ExternalInput")
    madj_d = nc.dram_tensor("madj", [G, N, N], bf16, kind="ExternalInput")
    kd1_d = nc.dram_tensor("kd1", [128, 2 * G], f32, kind="ExternalInput")
    esel_d = nc.dram_tensor("esel", [G, G * 128], bf16, kind="ExternalInput")
    w12_d = nc.dram_tensor("w12", [F_IN, H1], bf16, kind="ExternalInput")
    w22_d = nc.dram_tensor("w22", [H1, H1], bf16, kind="ExternalInput")
    w32_d = nc.dram_tensor("w32", [H1, H1], bf16, kind="ExternalInput")
    w1_d = nc.dram_tensor("w1", [2 * H1, H1], bf16, kind="ExternalInput")
    w2_d = nc.dram_tensor("w2", [H1, H2], bf16, kind="ExternalInput")
    w3_d = nc.dram_tensor("w3", [H2, C], bf16, kind="ExternalInput")
    pwc_d = nc.dram_tensor("pwc", [128, 6], bf16, kind="ExternalInput")
    b12_d = nc.dram_tensor("b12", [1, H1], bf16, kind="ExternalInput")
    b22_d = nc.dram_tensor("b22", [1, H1], bf16, kind="ExternalInput")
    b32_d = nc.dram_tensor("b32", [1, H1], bf16, kind="ExternalInput")
    b1_d = nc.dram_tensor("b1", [1, H1], bf16, kind="ExternalInput")
    b2_d = nc.dram_tensor("b2", [1, H2], bf16, kind="ExternalInput")
    b3_d = nc.dram_tensor("b3", [1, C], bf16, kind="ExternalInput")
    out_d = nc.dram_tensor("out", [G, C], f32, kind="ExternalOutput")

    with tile.TileContext(nc) as tc:
        import contextlib
        with contextlib.ExitStack() as ctx:
            big = ctx.enter_context(tc.tile_pool(name="big", bufs=1))
            sm = ctx.enter_context(tc.tile_pool(name="sm", bufs=1))
            hmp = ctx.enter_context(tc.tile_pool(name="hmp", bufs=4))
            pmm = ctx.enter_context(tc.tile_pool(name="pmm", bufs=5, space="PSUM"))
            pt = ctx.enter_context(tc.tile_pool(name="pt", bufs=1, space="PSUM"))
            pv = ctx.enter_context(tc.tile_pool(name="pv", bufs=2, space="PSUM"))

            ADJ = big.tile([128, 2 * GN], bf16, tag="adj")
            MADJ = big.tile([128, 2 * GN], bf16, tag="madj")
            XT = big.tile([128, GN], bf16, tag="xt")
            U = big.tile([128, 2 * GN], bf16, tag="u")
            U2 = big.tile([128, 2 * GN], bf16, tag="u2")
            # HRAW: (g, ft)-major bf16: chunk (g, ft) at (2g+ft)*N
            HRAW = big.tile([128, 2 * GN], bf16, tag="hraw")
            # CSSB2: cs broadcast, doubled to match HRAW's (g, ft) pairs
            CSSB2 = big.tile([128, 2 * GN], bf16, tag="cssb2")
            if with_bias:
                HMF = big.tile([128, 2 * GN], bf16, tag="hmf")

            ESEL = sm.tile([G, G * 128], bf16, tag="esel")
            W12S = sm.tile([128, H1], bf16, tag="w12s")
            W22S = sm.tile([128, 2 * H1], bf16, tag="w22s")
            W32S = sm.tile([128, 2 * H1], bf16, tag="w32s")
            W1S = sm.tile([128, 4 * H1], bf16, tag="w1s")
            W2S = sm.tile([128, 2 * H2], bf16, tag="w2s")
            W3S = sm.tile([128, C], bf16, tag="w3s")
            PWC = sm.tile([128, 6], bf16, tag="pwc")
            B12R = sm.tile([1, H1], bf16, tag="b12r")
            B22R = sm.tile([1, H1], bf16, tag="b22r")
            B32R = sm.tile([1, H1], bf16, tag="b32r")
            B1R = sm.tile([1, H1], bf16, tag="b1r")
            B2R = sm.tile([1, H2], bf16, tag="b2r")
            B3R = sm.tile([1, C], bf16, tag="b3r")
            BLV = {0: B12R, 1: B22R, 2: B32R}

            IDT = sm.tile([128, 128], f32, tag="idt")
            ONEB = sm.tile([1, 128], bf16, tag="oneb")
            EPSB = sm.tile([128, 1], f32, tag="epsb")

            # column tiles [128, 2G]: col = mt*G + g
            LNC = sm.tile([128, 2 * G], f32, tag="lnc")
            DICB = sm.tile([128, 2 * G], f32, tag="dicb")
            KVCA = sm.tile([128, 2 * G], f32, tag="kvca")
            KDCA = sm.tile([128, 2 * G], f32, tag="kdca")
            KD2CA = sm.tile([128, 2 * G], f32, tag="kd2ca")
            C1C = sm.tile([128, 2 * G], f32, tag="c1c")
            SCOL = sm.tile([128, 2 * G], f32, tag="scol")
            ECOL = sm.tile([128, 2 * G], f32, tag="ecol")
            DCOL = sm.tile([128, 2 * G], f32, tag="dcol")
            RCOL = sm.tile([128, 2 * G], f32, tag="rcol")
            THC = sm.tile([128, 2 * G], f32, tag="thc")
            CSC = sm.tile([128, 2 * G], f32, tag="csc")
            KVT = [sm.tile([128, G], bf16, tag=f"kvt{i}", name=f"KVT{i}")
                   for i in range(2)]

            # row tiles [16, 256] (one graph per partition)
            KV = sm.tile([16, N], f32, tag="kv")
            KVN = sm.tile([16, N], f32, tag="kvn")
            S = sm.tile([16, N], f32, tag="s")
            AM16 = sm.tile([16, N], f32, tag="am16")
            DS = sm.tile([16, N], f32, tag="ds")
            WRK = sm.tile([16, N], f32, tag="wrk")
            T1 = sm.tile([16, N], f32, tag="t1")
            CSB16 = sm.tile([16, N], bf16, tag="csb16")
            TK8 = sm.tile([16, 8], f32, tag="tk8")

            # readout accumulators: col = 2g + ft
            RDTX = [sm.tile([128, 2 * G], f32, tag=f"rdtx{l}",
                            name=f"RDTX{l}") for l in range(3)]
            RDTS = [sm.tile([128, 2 * G], f32, tag=f"rdts{l}",
                            name=f"RDTS{l}") for l in range(3)]
            ZACC = sm.tile([128, 64], f32, tag="zacc")
            SCRC = sm.tile([128, G], f32, tag="scrc")

            Z1 = sm.tile([16, H1], f32, tag="z1")
            Z1T = sm.tile([128, 2 * G], bf16, tag="z1t")
            Z2 = sm.tile([16, H2], f32, tag="z2")
            Z2T = sm.tile([128, G], bf16, tag="z2t")
            M16 = sm.tile([16, 1], f32, tag="m16")
            NM16 = sm.tile([16, 1], f32, tag="nm16")
            ES = sm.tile([16, C], f32, tag="es")
            SE = sm.tile([16, 1], f32, tag="se")
            LSE = sm.tile([16, 1], f32, tag="lse")
            OUTS = sm.tile([16, C], f32, tag="outs")

            def usl(g, t):  # U/U2 column slice for (graph, node-half)
                o = (g * 2 + t) * N
                return slice(o, o + N)

            def asl(g, st):  # ADJ/MADJ block (graph, src-half): [s128, d256]
                o = (g * 2 + st) * N
                return slice(o, o + N)

            def col(g, mt):
                return slice(mt * G + g, mt * G + g + 1)

            def hch(g, ft):  # HRAW chunk slice for (graph, feat-half)
                o = (2 * g + ft) * N
                return slice(o, o + N)

            def csl(g):
                return slice(g * N, (g + 1) * N)

            # ---- consts + input DMAs
            make_identity(nc, IDT[:])
            nc.gpsimd.memset(ONEB[:], 1.0)
            nc.gpsimd.memset(EPSB[:], 1e-12)
            nc.gpsimd.memset(KV[:], 1.0)

            nc.sync.dma_start(XT[:], xt_d[:])
            nc.sync.dma_start(W12S[:], w12_d[:])
            nc.sync.dma_start(KDCA[:], kd1_d[:])
            nc.sync.dma_start(PWC[:], pwc_d[:])
            # MADJ in 4-graph chunks so level-1 fused prop can start early
            for c4 in range(4):
                gs = slice(c4 * 4, c4 * 4 + 4)
                nc.sync.dma_start(
                    MADJ[:, c4 * 2048:(c4 + 1) * 2048].rearrange(
                        "p (g t d) -> p g t d", g=4, t=2),
                    madj_d[gs].rearrange("g (t p) d -> p g t d", p=128))
            for c4 in range(4):
                gs = slice(c4 * 4, c4 * 4 + 4)
                nc.sync.dma_start(
                    ADJ[:, c4 * 2048:(c4 + 1) * 2048].rearrange(
                        "p (g t d) -> p g t d", g=4, t=2),
                    adj_d[gs].rearrange("g (t p) d -> p g t d", p=128))
            nc.sync.dma_start(ESEL[:], esel_d[:])
            nc.sync.dma_start(W22S[:].rearrange("p (t n) -> p t n", n=H1),
                              w22_d[:].rearrange("(t p) n -> p t n", p=128))
            nc.sync.dma_start(W32S[:].rearrange("p (t n) -> p t n", n=H1),
                              w32_d[:].rearrange("(t p) n -> p t n", p=128))
            nc.sync.dma_start(W1S[:].rearrange("p (t n) -> p t n", n=H1),
                              w1_d[:].rearrange("(t p) n -> p t n", p=128))
            nc.sync.dma_start(W2S[:].rearrange("p (t n) -> p t n", n=H2),
                              w2_d[:].rearrange("(t p) n -> p t n", p=128))
            nc.sync.dma_start(W3S[:], w3_d[:])
            for dst, src in ((B12R, b12_d), (B22R, b22_d), (B32R, b32_d),
                             (B1R, b1_d), (B2R, b2_d), (B3R, b3_d)):
                nc.sync.dma_start(dst[:], src[:])

            def deg_c1():
                """deg cols = W^T kv -> dinv via exp(-ln/2) -> kd/kd2/c1."""
                pdg = pv.tile([128, 2 * G], f32, tag="pcol")
                for g in range(G):
                    for dh in range(2):
                        for st in range(2):
                            ao = (g * 2 + st) * N + dh * 128
                            nc.tensor.matmul(pdg[:, col(g, dh)],
                                             ADJ[:, ao:ao + 128],
                                             KVT[st][:, g:g + 1],
                                             start=(st == 0), stop=(st == 1))
                if use_expln:
                    nc.scalar.activation(LNC[:], pdg[:], AF.Ln,
                                         bias=EPSB[:, 0:1])
                    nc.scalar.activation(DICB[:], LNC[:], AF.Exp, scale=-0.5)
                else:
                    nc.scalar.activation(LNC[:], pdg[:], AF.Sqrt,
                                         bias=EPSB[:, 0:1])
                    nc.vector.reciprocal(DICB[:], LNC[:])
                nc.vector.tensor_mul(KDCA[:], DICB[:], KVCA[:])
                nc.vector.tensor_mul(KD2CA[:], KDCA[:], DICB[:])
                nc.vector.tensor_mul(C1C[:], KDCA[:], CSC[:])

            def dense(lvl):
                """U = scale o relu(h @ W), node-major; stationary = h."""
                if lvl == 0:
                    WS, kts = W12S, 1
                else:
                    WS = {1: W22S, 2: W32S}[lvl]
                    kts = 2
                for g in range(G):
                    for mt in range(2):
                        ps = pmm.tile([128, H1], f32, tag="ps")
                        for kt in range(kts):
                            if lvl == 0:
                                lhs = XT[:, g * N + mt * 128:
                                         g * N + mt * 128 + 128]
                            elif with_bias:
                                lhs = HMF[:, (2 * g + kt) * N + mt * 128:
                                          (2 * g + kt) * N + mt * 128 + 128]
                            else:
                                lhs = HRAW[:, (2 * g + kt) * N + mt * 128:
                                           (2 * g + kt) * N + mt * 128 + 128]
                            nc.tensor.matmul(ps[:], lhs,
                                             WS[:, kt * H1:(kt + 1) * H1],
                                             start=(kt == 0),
                                             stop=(not with_bias and
                                                   kt == kts - 1))
                        if with_bias:
                            nc.tensor.matmul(ps[:], ONEB[0:1, :], BLV[lvl][:],
                                             start=False, stop=True)
                        sc = KDCA if (lvl == 0 or with_bias) else C1C
                        dst = U[:, usl(g, mt)]
                        if g % 2 == 0:
                            nc.scalar.activation(dst, ps[:], AF.Relu,
                                                 scale=sc[:, col(g, mt)])
                        else:
                            nc.vector.tensor_scalar(dst, ps[:],
                                                    sc[:, col(g, mt)], 0.0,
                                                    op0=OP.mult, op1=OP.max)

            def hop_out(g, AD, UIN):
                """Feature-major 2-MM-group hop out of AD with stationary
                chunks of UIN; drains raw into HRAW pair slice."""
                if split_psum:
                    for ft in range(2):
                        ps = pmm.tile([128, H1], f32, tag="ps")
                        for eh in range(2):
                            uo = (g * 2 + eh) * N + ft * 128
                            nc.tensor.matmul(ps[:], UIN[:, uo:uo + 128],
                                             AD[:, asl(g, eh)],
                                             start=(eh == 0), stop=(eh == 1))
                        dst = HRAW[:, hch(g, ft)]
                        if (2 * g + ft) % 2 == 0:
                            nc.scalar.copy(dst, ps[:])
                        else:
                            nc.vector.tensor_copy(dst, ps[:])
                else:
                    P = pmm.tile([128, 2 * H1], f32, tag="ps")
                    for ft in range(2):
                        for eh in range(2):
                            uo = (g * 2 + eh) * N + ft * 128
                            nc.tensor.matmul(P[:, ft * H1:(ft + 1) * H1],
                                             UIN[:, uo:uo + 128],
                                             AD[:, asl(g, eh)],
                                             start=(eh == 0), stop=(eh == 1))
                    dst = HRAW[:, 2 * g * N:2 * g * N + 2 * N]
                    if g % 2 == 0:
                        nc.scalar.copy(dst, P[:])
                    else:
                        nc.vector.tensor_copy(dst, P[:])

            def fused_prop1():
                # p2 = (W D^2 W)^T-contracted in one pass: stationary = U
                for g in range(G):
                    hop_out(g, MADJ, U)

            def prop23():
                # hop1: u2 = kd2 o (W^T u), node-major
                for g in range(G):
                    for dh in range(2):
                        ps = pmm.tile([128, H1], f32, tag="ps")
                        for st in range(2):
                            ao = (g * 2 + st) * N + dh * 128
                            nc.tensor.matmul(ps[:], ADJ[:, ao:ao + 128],
                                             U[:, usl(g, st)],
                                             start=(st == 0), stop=(st == 1))
                        dst = U2[:, usl(g, dh)]
                        if g % 2 == 0:
                            nc.scalar.activation(dst, ps[:], AF.Copy,
                                                 scale=KD2CA[:, col(g, dh)])
                        else:
                            nc.vector.tensor_scalar_mul(dst, ps[:],
                                                        KD2CA[:, col(g, dh)])
                # hop2: p2 = W^T u2, FEATURE-major; raw drain to HRAW
                for g in range(G):
                    hop_out(g, ADJ, U2)

            def trow(dst_row, src_col_ap, mt, out_bf=False):
                """[128, G] column-tile slice -> row-tile [16, 128] block."""
                pp = pt.tile([128, 128], f32, tag="pst")
                nc.tensor.transpose(pp[0:16, :], src_col_ap, IDT[:])
                eng = nc.scalar.copy if out_bf else nc.vector.tensor_copy
                eng(dst_row[0:16, mt * 128:(mt + 1) * 128], pp[0:16, :])

            def score(lvl):
                """score cols s = kd o (pw . p2); tanh via exp identity."""
                psc = pv.tile([128, 2 * G], f32, tag="pcol")
                for g in range(G):
                    for mt in range(2):
                        for ft in range(2):
                            ho = (2 * g + ft) * N + mt * 128
                            nc.tensor.matmul(
                                psc[:, col(g, mt)],
                                HRAW[:, ho:ho + 128],
                                PWC[:, lvl * 2 + ft:lvl * 2 + ft + 1],
                                start=(ft == 0), stop=(ft == 1))
                nc.vector.tensor_mul(SCOL[:], psc[:], KDCA[:])
                nc.scalar.activation(THC[:], SCOL[:], AF.Tanh)
                for mt in range(2):
                    trow(S, SCOL[:, mt * G:(mt + 1) * G], mt)

            def topk_pool(lvl):
                d = DROPS[lvl]
                # mask inactive scores; drop-side top-k
                nc.vector.tensor_scalar(AM16[:], KV[:], 1.0, BIG,
                                        op0=OP.subtract, op1=OP.mult)
                nc.vector.tensor_sub(DS[:], AM16[:], S[:])
                cur = DS
                for it in range((d + 7) // 8):
                    nc.vector.max(TK8[:], cur[:])
                    rem = d - it * 8
                    if rem < 8:
                        nc.vector.memset(TK8[:, rem:8], MINV)
                    nc.vector.match_replace(WRK[:], TK8[:], cur[:], MINV)
                    cur = WRK
                # kv_new: 1 where WRK is a kept score (-s), 0 elsewhere
                nc.vector.tensor_scalar(T1[:], WRK[:], 1e-29, 1.0,
                                        op0=OP.mult, op1=OP.add)
                nc.vector.tensor_scalar(KVN[:], T1[:], 0.0, 1.0,
                                        op0=OP.max, op1=OP.min)
                nc.vector.tensor_copy(KV[:], KVN[:])
                # kv columns (fp32 + bf16) for next level's deg
                for mt in range(2):
                    pp = pt.tile([128, 128], f32, tag="pst")
                    nc.tensor.transpose(pp[:, 0:16],
                                        KVN[0:16, mt * 128:(mt + 1) * 128],
                                        IDT[0:16, 0:16])
                    nc.scalar.copy(KVCA[:, mt * G:(mt + 1) * G], pp[:, 0:16])
                    nc.vector.tensor_copy(KVT[mt][:], pp[:, 0:16])
                # cs = kd*kv_new*tanh(s) as columns -> bf16 rows
                nc.vector.tensor_mul(CSC[:], KDCA[:], KVCA[:])
                nc.vector.tensor_mul(CSC[:], CSC[:], THC[:])
                for mt in range(2):
                    trow(CSB16, CSC[:, mt * G:(mt + 1) * G], mt, out_bf=True)
                # selector broadcast rows -> psum pair -> SBUF bf16 (doubled)
                for g in range(G):
                    cb = pmm.tile([128, 2 * N], f32, tag="ps")
                    for half in range(2):
                        nc.tensor.matmul(cb[:, half * N:(half + 1) * N],
                                         ESEL[:, g * 128:(g + 1) * 128],
                                         CSB16[:], start=True, stop=True)
                    dst = CSSB2[:, 2 * g * N:2 * g * N + 2 * N]
                    if g % 2 == 0:
                        nc.scalar.copy(dst, cb[:])
                    else:
                        nc.vector.tensor_copy(dst, cb[:])

            def readout(lvl):
                """GpSimd pair products; DVE max pair-reduce + 4x sum-accum.
                Max seeds at 0 implicitly: dropped nodes contribute cs=0."""
                for g in range(G):
                    if with_bias:
                        hm2 = HMF[:, 2 * g * N:2 * g * N + 2 * N]
                    else:
                        hmt = hmp.tile([128, 2 * N], bf16, tag="hm")
                        hm2 = hmt[:]
                    nc.vector.tensor_mul(hm2, HRAW[:, 2 * g * N:
                                                    2 * g * N + 2 * N],
                                         CSSB2[:, 2 * g * N:
                                               2 * g * N + 2 * N])
                    nc.vector.tensor_reduce(
                        RDTX[lvl][:, 2 * g:2 * g + 2],
                        hm2.rearrange("p (f n) -> p f n", f=2),
                        axis=AX.X, op=OP.max)
                    for ft in range(2):
                        rc = 2 * g + ft
                        sdum = hmp.tile([128, N], bf16, tag="sdum")
                        nc.scalar.activation(
                            sdum[:], hm2[:, ft * N:(ft + 1) * N], AF.Copy,
                            accum_out=RDTS[lvl][:, rc:rc + 1])

            # ---- the network
            dense(0)
            fused_prop1()
            score(0)
            topk_pool(0)
            readout(0)
            for lvl in range(1, 3):
                deg_c1()
                dense(lvl)
                prop23()
                score(lvl)
                topk_pool(lvl)
                readout(lvl)

            # ---- combine readouts: z = sum_lvl [max | mean/k]
            def ftview(t, ft):
                # [128, 2G] (g, ft)-major -> [128, 1, G] slice for this ft
                return t[:].rearrange("p (g f) -> p f g", f=2)[:, ft:ft + 1, :]

            for kind in range(2):
                RD = RDTX if kind == 0 else RDTS
                for ft in range(2):
                    cg = (kind * 2 + ft) * G
                    dst = ZACC[:, cg:cg + G].rearrange("p (f g) -> p f g",
                                                       f=1)
                    v0, v1, v2 = (ftview(RD[l], ft) for l in range(3))
                    if kind == 0:
                        nc.vector.tensor_add(dst, v0, v1)
                        nc.vector.tensor_add(dst, dst, v2)
                    else:
                        nc.vector.tensor_scalar_mul(dst, v0, 1.0 / KS[0])
                        for l2, vv in ((1, v1), (2, v2)):
                            s3 = SCRC[:].rearrange("p (f g) -> p f g", f=1)
                            nc.vector.tensor_scalar_mul(s3, vv, 1.0 / KS[l2])
                            nc.vector.tensor_add(dst, dst, s3)

            # ---- final MLP + log_softmax
            ZB = sm.tile([128, 64], bf16, tag="zb")
            nc.vector.tensor_copy(ZB[:], ZACC[:])
            ps1 = pv.tile([16, H1], f32, tag="pcol")
            for kt in range(4):
                nc.tensor.matmul(ps1[0:16, :], ZB[:, kt * 16:(kt + 1) * 16],
                                 W1S[:, kt * H1:(kt + 1) * H1],
                                 start=(kt == 0), stop=False)
            nc.tensor.matmul(ps1[0:16, :], ONEB[0:1, 0:16], B1R[:],
                             start=False, stop=True)
            nc.scalar.activation(Z1[:], ps1[0:16, :], AF.Relu)
            for kt in range(2):
                pp = pt.tile([128, 128], f32, tag="pst")
                nc.tensor.transpose(pp[:, 0:16],
                                    Z1[0:16, kt * 128:(kt + 1) * 128],
                                    IDT[0:16, 0:16])
                nc.scalar.copy(Z1T[:, kt * G:(kt + 1) * G], pp[:, 0:16])
            ps2 = pv.tile([16, H2], f32, tag="pcol")
            for kt in range(2):
                nc.tensor.matmul(ps2[0:16, :], Z1T[:, kt * G:(kt + 1) * G],
                                 W2S[:, kt * H2:(kt + 1) * H2],
                                 start=(kt == 0), stop=False)
            nc.tensor.matmul(ps2[0:16, :], ONEB[0:1, 0:16], B2R[:],
                             start=False, stop=True)
            nc.scalar.activation(Z2[:], ps2[0:16, :], AF.Relu)
            pp = pt.tile([128, 128], f32, tag="pst")
            nc.tensor.transpose(pp[:, 0:16], Z2[0:16, :], IDT[0:16, 0:16])
            nc.scalar.copy(Z2T[:], pp[:, 0:16])
            ps3 = pv.tile([16, C], f32, tag="pcol")
            nc.tensor.matmul(ps3[0:16, :], Z2T[:], W3S[:], start=True,
                             stop=False)
            nc.tensor.matmul(ps3[0:16, :], ONEB[0:1, 0:16], B3R[:],
                             start=False, stop=True)
            nc.vector.tensor_reduce(M16[:], ps3[0:16, :], axis=AX.X, op=OP.max)
            nc.vector.tensor_scalar_mul(NM16[:], M16[:], -1.0)
            nc.scalar.activation(ES[:], ps3[0:16, :], AF.Exp,
                                 bias=NM16[0:16, 0:1], scale=1.0)
            nc.vector.tensor_reduce(SE[:], ES[:], axis=AX.X, op=OP.add)
            nc.scalar.activation(LSE[:], SE[:], AF.Ln)
            nc.vector.tensor_scalar(OUTS[:], ps3[0:16, :], M16[0:16, 0:1],
                                    LSE[0:16, 0:1], op0=OP.subtract,
                                    op1=OP.subtract)
            nc.sync.dma_start(out_d[:], OUTS[:])

    nc.compile()
    return nc


def _get_nc(with_bias, **kw):
    key = f"nc{int(with_bias)}{sorted(kw.items())}"
    if key not in _CACHE:
        _CACHE[key] = _build(with_bias, **kw)
    return _CACHE[key]


def _host_prep(inputs):
    import ml_dtypes
    bfd = ml_dtypes.bfloat16
    x = np.asarray(inputs["x"], np.float32)
    edges = np.asarray(inputs["edges"], np.int32)
    src = edges[..., 0].astype(np.int64)
    dst = edges[..., 1].astype(np.int64)
    gidx = np.arange(B, dtype=np.int64)[:, None]
    flat = (gidx * N * N + src * N + dst).ravel()
    A = np.bincount(flat, minlength=B * N * N).astype(np.float32)
    A = A.reshape(B, N, N)
    A += np.eye(N, dtype=np.float32)[None]

    # level-1 norms (keep-mask all ones) + fused 2-hop matrix
    degk = A.sum(axis=1)                       # [B, N]: col sums of W
    dinv1 = degk ** -0.5
    MADJ = np.matmul(A * (dinv1 ** 2)[:, None, :], A)  # (W D^2 W)[e, d]

    Ab = A.astype(bfd)
    Mb = MADJ.astype(bfd)
    xt = np.ascontiguousarray(
        x.reshape(NCORES, G * N, F_IN).transpose(0, 2, 1)).astype(bfd)

    # kd1 column tiles per core: [128, 2G], col = mt*G + g
    kd1 = np.zeros((NCORES, 128, 2 * G), np.float32)
    for c in range(NCORES):
        dv = dinv1[c * G:(c + 1) * G]          # [G, N]
        for mt in range(2):
            kd1[c, :, mt * G:(mt + 1) * G] = dv[:, mt * 128:(mt + 1) * 128].T

    esel = np.zeros((G, G * 128), np.float32)
    for g in range(G):
        esel[g, g * 128:(g + 1) * 128] = 1.0

    shared = {"esel": esel.astype(bfd)}
    for name, key in (("w12", "W12"), ("w22", "W22"), ("w32", "W32"),
                      ("w1", "W1"), ("w2", "W2"), ("w3", "W3")):
        shared[name] = np.ascontiguousarray(
            np.asarray(inputs[key], np.float32).astype(bfd))
    for name, key, n in (("b12", "b12", H1), ("b22", "b22", H1),
                         ("b32", "b32", H1), ("b1", "b1", H1),
                         ("b2", "b2", H2), ("b3", "b3", C)):
        shared[name] = np.asarray(inputs[key], np.float32).reshape(1, n) \
            .astype(bfd)
    pwc = np.zeros((128, 6), np.float32)
    for i, key in enumerate(("pw1", "pw2", "pw3")):
        pw = np.asarray(inputs[key], np.float32)
        pwn = pw / np.linalg.norm(pw)
        pwc[:, 2 * i] = pwn[:128]
        pwc[:, 2 * i + 1] = pwn[128:]
    shared["pwc"] = pwc.astype(bfd)

    with_bias = any(np.any(np.asarray(inputs[k])) for k in
                    ("b12", "b22", "b32"))
    in_maps = []
    for c in range(NCORES):
        m = dict(shared)
        m["xt"] = np.ascontiguousarray(xt[c])
        m["adj"] = np.ascontiguousarray(Ab[c * G:(c + 1) * G])
        m["madj"] = np.ascontiguousarray(Mb[c * G:(c + 1) * G])
        m["kd1"] = np.ascontiguousarray(kd1[c])
        in_maps.append(m)
    return in_maps, with_bias


def kernel(**inputs):
    from concourse.bass_utils import run_bass_kernel_spmd
    in_maps, with_bias = _host_prep(inputs)
    nc = _get_nc(with_bias)
    r = run_bass_kernel_spmd(nc, in_maps, core_ids=list(range(NCORES)))
    out = np.concatenate([r.results[c]["out"] for c in range(NCORES)], axis=0)
    return out.astype(np.float32)


def run_traced(inputs):
    """Like kernel() but with NTFF tracing; returns (out, BassKernelResults)."""
    import sys
    import types
    if "antenv.axon_hooks" not in sys.modules:
        hooks = types.ModuleType("antenv.axon_hooks")
        hooks._hook = None
        hooks.set_axon_ntff_profile_hook = lambda h: setattr(hooks, "_hook", h)
        hooks.get_axon_ntff_profile_hook = lambda: hooks._hook
        sys.modules["antenv.axon_hooks"] = hooks
        from trn_agent_boot.trn_boot import _ntff_profile_via_ctypes
        hooks.set_axon_ntff_profile_hook(
            _ntff_profile_via_ctypes("/opt/axon/libaxon_pjrt.so"))
    from concourse.bass_utils import run_bass_kernel_spmd
    in_maps, with_bias = _host_prep(inputs)
    nc = _get_nc(with_bias)
    r = run_bass_kernel_spmd(nc, in_maps, core_ids=list(range(NCORES)),
                             trace=True)
    out = np.concatenate([r.results[c]["out"] for c in range(NCORES)], axis=0)
    return out.astype(np.float32), r


# revision 24
# speedup vs baseline: 1.0457x; 1.0143x over previous
"""Trainium2 Bass kernel for nn_ConvG (3-level GCN + TopK pooling + readout).

Data-parallel over 8 NeuronCores (16 graphs each). Host converts the edge
list to dense per-graph adjacency W = A + I (bf16), pre-transposes x, and
additionally precomputes for level 1 (where the keep-mask is all-ones):
  - dinv1 = (col-sums of W)^-1/2 as ready-made scale columns, and
  - MADJ = W @ diag(dinv1^2) @ W, which fuses the two propagation hops of
    level 1 into a single dense matmul pass.

Key structural points vs. a naive pipeline:
  - The TopK pool scale cs = kd*kv*tanh(s) is NOT materialized into the
    feature map. The next dense layer consumes raw HRAW with a folded
    per-node drain scale c1 = kd_next*cs (valid since kd_next >= 0:
    kd*relu(cs*z) == relu(kd*cs*z)), so no pooled copy of h exists.
  - Readouts come from two fused DVE tensor_tensor_reduce ops per
    (graph, feature-half): product cs*HRAW with max-accum (seeded at 0 --
    dropped nodes contribute exactly 0, and HRAW >= 0 up to tiny relu
    zeros, so seeding at 0 matches the reference max to ~3e-4 absolute)
    and with sum-accum (exact: dropped nodes contribute cs = 0).
  - All transcendentals use one activation-table set
    (natural_log_exp_and_others): dinv = exp(-0.5*ln(deg)), tanh computed
    as 1 - 2/(exp(2s)+1), and the final log_softmax's Exp/Ln.
  - hop2 / fused-hop psum pairs drain 512 wide; score columns multiply
    kd directly from PSUM; level boundaries keep per-graph slice deps so
    Tile can pipeline graphs across stages.
"""
import numpy as np

G = 16            # graphs per core
N = 256           # nodes per graph
F_IN = 128
H1 = 256
H2 = 128
C = 10
NCORES = 8
B = G * NCORES    # 128 graphs
KS = [205, 164, 132]
DROPS = [51, 41, 32]
BIG = 1e30
MINV = -1e30

_CACHE = {}


def _build(with_bias, split_psum=False, use_ttr=False, use_expln=True):
    import concourse.bacc as bacc
    import concourse.mybir as mybir
    import concourse.tile as tile
    from concourse.masks import make_identity

    f32 = mybir.dt.float32
    bf16 = mybir.dt.bfloat16
    AF = mybir.ActivationFunctionType
    OP = mybir.AluOpType
    AX = mybir.AxisListType

    nc = bacc.Bacc("TRN2", target_bir_lowering=False, debug=False)

    GN = G * N  # 4096

    xt_d = nc.dram_tensor("xt", [F_IN, GN], bf16, kind="ExternalInput")
    adj_d = nc.dram_tensor("adj", [G, N, N], bf16, kind="ExternalInput")
    madj_d = nc.dram_tensor("madj", [G, N, N], bf16, kind="ExternalInput")
    kd1_d = nc.dram_tensor("kd1", [128, 2 * G], f32, kind="ExternalInput")
    esel_d = nc.dram_tensor("esel", [G, G * 128], bf16, kind="ExternalInput")
    w12_d = nc.dram_tensor("w12", [F_IN, H1], bf16, kind="ExternalInput")
    w22_d = nc.dram_tensor("w22", [H1, H1], bf16, kind="ExternalInput")
    w32_d = nc.dram_tensor("w32", [H1, H1], bf16, kind="ExternalInput")
    w1_d = nc.dram_tensor("w1", [2 * H1, H1], bf16, kind="ExternalInput")
    w2_d = nc.dram_tensor("w2", [H1, H2], bf16, kind="ExternalInput")
    w3_d = nc.dram_tensor("w3", [H2, C], bf16, kind="ExternalInput")
    pwc_d = nc.dram_tensor("pwc", [128, 6], bf16, kind="ExternalInput")
    b12_d = nc.dram_tensor("b12", [1, H1], bf16, kind="ExternalInput")
    b22_d = nc.dram_tensor("b22", [1, H1], bf16, kind="ExternalInput")
    b32_d = nc.dram_tensor("b32", [1, H1], bf16, kind="ExternalInput")
    b1_d = nc.dram_tensor("b1", [1, H1], bf16, kind="ExternalInput")
    b2_d = nc.dram_tensor("b2", [1, H2], bf16, kind="ExternalInput")
    b3_d = nc.dram_tensor("b3", [1, C], bf16, kind="ExternalInput")
    out_d = nc.dram_tensor("out", [G, C], f32, kind="ExternalOutput")

    with tile.TileContext(nc) as tc:
        import contextlib
        with contextlib.ExitStack() as ctx:
            big = ctx.enter_context(tc.tile_pool(name="big", bufs=1))
            sm = ctx.enter_context(tc.tile_pool(name="sm", bufs=1))
            hmp = ctx.enter_context(tc.tile_pool(name="hmp", bufs=4))
            pmm = ctx.enter_context(tc.tile_pool(name="pmm", bufs=5, space="PSUM"))
            pt = ctx.enter_context(tc.tile_pool(name="pt", bufs=1, space="PSUM"))
            pv = ctx.enter_context(tc.tile_pool(name="pv", bufs=2, space="PSUM"))

            ADJ = big.tile([128, 2 * GN], bf16, tag="adj")
            MADJ = big.tile([128, 2 * GN], bf16, tag="madj")
            XT = big.tile([128, GN], bf16, tag="xt")
            U = big.tile([128, 2 * GN], bf16, tag="u")
            U2 = big.tile([128, 2 * GN], bf16, tag="u2")
            # HRAW: (g, ft)-major bf16: chunk (g, ft) at (2g+ft)*N
            HRAW = big.tile([128, 2 * GN], bf16, tag="hraw")
            # CSSB2: cs broadcast, doubled to match HRAW's (g, ft) pairs
            CSSB2 = big.tile([128, 2 * GN], bf16, tag="cssb2")
            if with_bias:
                HMF = big.tile([128, 2 * GN], bf16, tag="hmf")

            ESEL = sm.tile([G, G * 128], bf16, tag="esel")
            W12S = sm.tile([128, H1], bf16, tag="w12s")
            W22S = sm.tile([128, 2 * H1], bf16, tag="w22s")
            W32S = sm.tile([128, 2 * H1], bf16, tag="w32s")
            W1S = sm.tile([128, 4 * H1], bf16, tag="w1s")
            W2S = sm.tile([128, 2 * H2], bf16, tag="w2s")
            W3S = sm.tile([128, C], bf16, tag="w3s")
            PWC = sm.tile([128, 6], bf16, tag="pwc")
            B12R = sm.tile([1, H1], bf16, tag="b12r")
            B22R = sm.tile([1, H1], bf16, tag="b22r")
            B32R = sm.tile([1, H1], bf16, tag="b32r")
            B1R = sm.tile([1, H1], bf16, tag="b1r")
            B2R = sm.tile([1, H2], bf16, tag="b2r")
            B3R = sm.tile([1, C], bf16, tag="b3r")
            BLV = {0: B12R, 1: B22R, 2: B32R}

            IDT = sm.tile([128, 128], f32, tag="idt")
            ONEB = sm.tile([1, 128], bf16, tag="oneb")
            EPSB = sm.tile([128, 1], f32, tag="epsb")

            # column tiles [128, 2G]: col = mt*G + g
            LNC = sm.tile([128, 2 * G], f32, tag="lnc")
            DICB = sm.tile([128, 2 * G], f32, tag="dicb")
            KVCA = sm.tile([128, 2 * G], f32, tag="kvca")
            KDCA = sm.tile([128, 2 * G], f32, tag="kdca")
            KD2CA = sm.tile([128, 2 * G], f32, tag="kd2ca")
            C1C = sm.tile([128, 2 * G], f32, tag="c1c")
            SCOL = sm.tile([128, 2 * G], f32, tag="scol")
            ECOL = sm.tile([128, 2 * G], f32, tag="ecol")
            DCOL = sm.tile([128, 2 * G], f32, tag="dcol")
            RCOL = sm.tile([128, 2 * G], f32, tag="rcol")
            THC = sm.tile([128, 2 * G], f32, tag="thc")
            CSC = sm.tile([128, 2 * G], f32, tag="csc")
            KVT = [sm.tile([128, G], bf16, tag=f"kvt{i}", name=f"KVT{i}")
                   for i in range(2)]

            # row tiles [16, 256] (one graph per partition)
            KV = sm.tile([16, N], f32, tag="kv")
            KVN = sm.tile([16, N], f32, tag="kvn")
            S = sm.tile([16, N], f32, tag="s")
            AM16 = sm.tile([16, N], f32, tag="am16")
            DS = sm.tile([16, N], f32, tag="ds")
            WRK = sm.tile([16, N], f32, tag="wrk")
            T1 = sm.tile([16, N], f32, tag="t1")
            CSB16 = sm.tile([16, N], bf16, tag="csb16")
            TK8 = sm.tile([16, 8], f32, tag="tk8")

            # readout accumulators: col = 2g + ft
            RDTX = [sm.tile([128, 2 * G], f32, tag=f"rdtx{l}",
                            name=f"RDTX{l}") for l in range(3)]
            RDTS = [sm.tile([128, 2 * G], f32, tag=f"rdts{l}",
                            name=f"RDTS{l}") for l in range(3)]
            ZACC = sm.tile([128, 64], f32, tag="zacc")
            SCRC = sm.tile([128, G], f32, tag="scrc")

            Z1 = sm.tile([16, H1], f32, tag="z1")
            Z1T = sm.tile([128, 2 * G], bf16, tag="z1t")
            Z2 = sm.tile([16, H2], f32, tag="z2")
            Z2T = sm.tile([128, G], bf16, tag="z2t")
            M16 = sm.tile([16, 1], f32, tag="m16")
            NM16 = sm.tile([16, 1], f32, tag="nm16")
            ES = sm.tile([16, C], f32, tag="es")
            SE = sm.tile([16, 1], f32, tag="se")
            LSE = sm.tile([16, 1], f32, tag="lse")
            OUTS = sm.tile([16, C], f32, tag="outs")

            def usl(g, t):  # U/U2 column slice for (graph, node-half)
                o = (g * 2 + t) * N
                return slice(o, o + N)

            def asl(g, st):  # ADJ/MADJ block (graph, src-half): [s128, d256]
                o = (g * 2 + st) * N
                return slice(o, o + N)

            def col(g, mt):
                return slice(mt * G + g, mt * G + g + 1)

            def hch(g, ft):  # HRAW chunk slice for (graph, feat-half)
                o = (2 * g + ft) * N
                return slice(o, o + N)

            def csl(g):
                return slice(g * N, (g + 1) * N)

            # ---- consts + input DMAs
            make_identity(nc, IDT[:])
            nc.gpsimd.memset(ONEB[:], 1.0)
            nc.gpsimd.memset(EPSB[:], 1e-12)
            nc.gpsimd.memset(KV[:], 1.0)

            for c4 in range(4):
                cs4 = slice(c4 * GN // 4, (c4 + 1) * GN // 4)
                nc.sync.dma_start(XT[:, cs4], xt_d[:, cs4])
            nc.sync.dma_start(W12S[:], w12_d[:])
            nc.sync.dma_start(KDCA[:], kd1_d[:])
            nc.sync.dma_start(PWC[:], pwc_d[:])
            # MADJ in 4-graph chunks so level-1 fused prop can start early
            for c4 in range(4):
                gs = slice(c4 * 4, c4 * 4 + 4)
                nc.sync.dma_start(
                    MADJ[:, c4 * 2048:(c4 + 1) * 2048].rearrange(
                        "p (g t d) -> p g t d", g=4, t=2),
                    madj_d[gs].rearrange("g (t p) d -> p g t d", p=128))
            for c4 in range(4):
                gs = slice(c4 * 4, c4 * 4 + 4)
                nc.sync.dma_start(
                    ADJ[:, c4 * 2048:(c4 + 1) * 2048].rearrange(
                        "p (g t d) -> p g t d", g=4, t=2),
                    adj_d[gs].rearrange("g (t p) d -> p g t d", p=128))
            nc.sync.dma_start(ESEL[:], esel_d[:])
            nc.sync.dma_start(W22S[:].rearrange("p (t n) -> p t n", n=H1),
                              w22_d[:].rearrange("(t p) n -> p t n", p=128))
            nc.sync.dma_start(W32S[:].rearrange("p (t n) -> p t n", n=H1),
                              w32_d[:].rearrange("(t p) n -> p t n", p=128))
            nc.sync.dma_start(W1S[:].rearrange("p (t n) -> p t n", n=H1),
                              w1_d[:].rearrange("(t p) n -> p t n", p=128))
            nc.sync.dma_start(W2S[:].rearrange("p (t n) -> p t n", n=H2),
                              w2_d[:].rearrange("(t p) n -> p t n", p=128))
            nc.sync.dma_start(W3S[:], w3_d[:])
            for dst, src in ((B12R, b12_d), (B22R, b22_d), (B32R, b32_d),
                             (B1R, b1_d), (B2R, b2_d), (B3R, b3_d)):
                nc.sync.dma_start(dst[:], src[:])

            def deg_c1():
                """deg cols = W^T kv -> dinv via exp(-ln/2) -> kd/kd2/c1."""
                pdg = pv.tile([128, 2 * G], f32, tag="pcol")
                for g in range(G):
                    for dh in range(2):
                        for st in range(2):
                            ao = (g * 2 + st) * N + dh * 128
                            nc.tensor.matmul(pdg[:, col(g, dh)],
                                             ADJ[:, ao:ao + 128],
                                             KVT[st][:, g:g + 1],
                                             start=(st == 0), stop=(st == 1))
                if use_expln:
                    nc.scalar.activation(LNC[:], pdg[:], AF.Ln,
                                         bias=EPSB[:, 0:1])
                    nc.scalar.activation(DICB[:], LNC[:], AF.Exp, scale=-0.5)
                else:
                    nc.scalar.activation(LNC[:], pdg[:], AF.Sqrt,
                                         bias=EPSB[:, 0:1])
                    nc.vector.reciprocal(DICB[:], LNC[:])
                nc.vector.tensor_mul(KDCA[:], DICB[:], KVCA[:])
                nc.vector.tensor_mul(KD2CA[:], KDCA[:], DICB[:])
                nc.vector.tensor_mul(C1C[:], KDCA[:], CSC[:])

            def dense(lvl):
                """U = scale o relu(h @ W), node-major; stationary = h."""
                if lvl == 0:
                    WS, kts = W12S, 1
                else:
                    WS = {1: W22S, 2: W32S}[lvl]
                    kts = 2
                for g in range(G):
                    for mt in range(2):
                        ps = pmm.tile([128, H1], f32, tag="ps")
                        for kt in range(kts):
                            if lvl == 0:
                                lhs = XT[:, g * N + mt * 128:
                                         g * N + mt * 128 + 128]
                            elif with_bias:
                                lhs = HMF[:, (2 * g + kt) * N + mt * 128:
                                          (2 * g + kt) * N + mt * 128 + 128]
                            else:
                                lhs = HRAW[:, (2 * g + kt) * N + mt * 128:
                                           (2 * g + kt) * N + mt * 128 + 128]
                            nc.tensor.matmul(ps[:], lhs,
                                             WS[:, kt * H1:(kt + 1) * H1],
                                             start=(kt == 0),
                                             stop=(not with_bias and
                                                   kt == kts - 1))
                        if with_bias:
                            nc.tensor.matmul(ps[:], ONEB[0:1, :], BLV[lvl][:],
                                             start=False, stop=True)
                        sc = KDCA if (lvl == 0 or with_bias) else C1C
                        dst = U[:, usl(g, mt)]
                        if g % 2 == 0:
                            nc.scalar.activation(dst, ps[:], AF.Relu,
                                                 scale=sc[:, col(g, mt)])
                        else:
                            nc.vector.tensor_scalar(dst, ps[:],
                                                    sc[:, col(g, mt)], 0.0,
                                                    op0=OP.mult, op1=OP.max)

            def hop_out(g, AD, UIN):
                """Feature-major 2-MM-group hop out of AD with stationary
                chunks of UIN; drains raw into HRAW pair slice."""
                if split_psum:
                    for ft in range(2):
                        ps = pmm.tile([128, H1], f32, tag="ps")
                        for eh in range(2):
                            uo = (g * 2 + eh) * N + ft * 128
                            nc.tensor.matmul(ps[:], UIN[:, uo:uo + 128],
                                             AD[:, asl(g, eh)],
                                             start=(eh == 0), stop=(eh == 1))
                        dst = HRAW[:, hch(g, ft)]
                        if (2 * g + ft) % 2 == 0:
                            nc.scalar.copy(dst, ps[:])
                        else:
                            nc.vector.tensor_copy(dst, ps[:])
                else:
                    P = pmm.tile([128, 2 * H1], f32, tag="ps")
                    for ft in range(2):
                        for eh in range(2):
                            uo = (g * 2 + eh) * N + ft * 128
                            nc.tensor.matmul(P[:, ft * H1:(ft + 1) * H1],
                                             UIN[:, uo:uo + 128],
                                             AD[:, asl(g, eh)],
                                             start=(eh == 0), stop=(eh == 1))
                    dst = HRAW[:, 2 * g * N:2 * g * N + 2 * N]
                    if g % 2 == 0:
                        nc.scalar.copy(dst, P[:])
                    else:
                        nc.vector.tensor_copy(dst, P[:])

            def fused_prop1():
                # p2 = (W D^2 W)^T-contracted in one pass: stationary = U
                for g in range(G):
                    hop_out(g, MADJ, U)

            def prop23():
                # hop1: u2 = kd2 o (W^T u), node-major
                for g in range(G):
                    for dh in range(2):
                        ps = pmm.tile([128, H1], f32, tag="ps")
                        for st in range(2):
                            ao = (g * 2 + st) * N + dh * 128
                            nc.tensor.matmul(ps[:], ADJ[:, ao:ao + 128],
                                             U[:, usl(g, st)],
                                             start=(st == 0), stop=(st == 1))
                        dst = U2[:, usl(g, dh)]
                        if g % 2 == 0:
                            nc.scalar.activation(dst, ps[:], AF.Copy,
                                                 scale=KD2CA[:, col(g, dh)])
                        else:
                            nc.vector.tensor_scalar_mul(dst, ps[:],
                                                        KD2CA[:, col(g, dh)])
                # hop2: p2 = W^T u2, FEATURE-major; raw drain to HRAW
                for g in range(G):
                    hop_out(g, ADJ, U2)

            def trow(dst_row, src_col_ap, mt, out_bf=False):
                """[128, G] column-tile slice -> row-tile [16, 128] block."""
                pp = pt.tile([128, 128], f32, tag="pst")
                nc.tensor.transpose(pp[0:16, :], src_col_ap, IDT[:])
                eng = nc.scalar.copy if out_bf else nc.vector.tensor_copy
                eng(dst_row[0:16, mt * 128:(mt + 1) * 128], pp[0:16, :])

            def score(lvl):
                """score cols s = kd o (pw . p2); tanh via exp identity."""
                psc = pv.tile([128, 2 * G], f32, tag="pcol")
                for g in range(G):
                    for mt in range(2):
                        for ft in range(2):
                            ho = (2 * g + ft) * N + mt * 128
                            nc.tensor.matmul(
                                psc[:, col(g, mt)],
                                HRAW[:, ho:ho + 128],
                                PWC[:, lvl * 2 + ft:lvl * 2 + ft + 1],
                                start=(ft == 0), stop=(ft == 1))
                nc.vector.tensor_mul(SCOL[:], psc[:], KDCA[:])
                nc.scalar.activation(THC[:], SCOL[:], AF.Tanh)
                for mt in range(2):
                    trow(S, SCOL[:, mt * G:(mt + 1) * G], mt)

            def topk_pool(lvl):
                d = DROPS[lvl]
                # mask inactive scores; drop-side top-k
                nc.vector.tensor_scalar(AM16[:], KV[:], 1.0, BIG,
                                        op0=OP.subtract, op1=OP.mult)
                nc.vector.tensor_sub(DS[:], AM16[:], S[:])
                cur = DS
                for it in range((d + 7) // 8):
                    nc.vector.max(TK8[:], cur[:])
                    rem = d - it * 8
                    if rem < 8:
                        nc.vector.memset(TK8[:, rem:8], MINV)
                    nc.vector.match_replace(WRK[:], TK8[:], cur[:], MINV)
                    cur = WRK
                # kv_new: 1 where WRK is a kept score (-s), 0 elsewhere
                nc.vector.tensor_scalar(T1[:], WRK[:], 1e-29, 1.0,
                                        op0=OP.mult, op1=OP.add)
                nc.vector.tensor_scalar(KVN[:], T1[:], 0.0, 1.0,
                                        op0=OP.max, op1=OP.min)
                nc.vector.tensor_copy(KV[:], KVN[:])
                # kv columns (fp32 + bf16) for next level's deg
                for mt in range(2):
                    pp = pt.tile([128, 128], f32, tag="pst")
                    nc.tensor.transpose(pp[:, 0:16],
                                        KVN[0:16, mt * 128:(mt + 1) * 128],
                                        IDT[0:16, 0:16])
                    nc.scalar.copy(KVCA[:, mt * G:(mt + 1) * G], pp[:, 0:16])
                    nc.vector.tensor_copy(KVT[mt][:], pp[:, 0:16])
                # cs = kd*kv_new*tanh(s) as columns -> bf16 rows
                nc.vector.tensor_mul(CSC[:], KDCA[:], KVCA[:])
                nc.vector.tensor_mul(CSC[:], CSC[:], THC[:])
                for mt in range(2):
                    trow(CSB16, CSC[:, mt * G:(mt + 1) * G], mt, out_bf=True)

            def bcast(lvl):
                # selector broadcast rows -> psum pair -> SBUF bf16 (doubled)
                for g in range(G):
                    cb = pmm.tile([128, 2 * N], f32, tag="ps")
                    for half in range(2):
                        nc.tensor.matmul(cb[:, half * N:(half + 1) * N],
                                         ESEL[:, g * 128:(g + 1) * 128],
                                         CSB16[:], start=True, stop=True)
                    dst = CSSB2[:, 2 * g * N:2 * g * N + 2 * N]
                    if g % 2 == 0:
                        nc.scalar.copy(dst, cb[:])
                    else:
                        nc.vector.tensor_copy(dst, cb[:])

            def readout(lvl):
                """GpSimd pair products; DVE max pair-reduce + 4x sum-accum.
                Max seeds at 0 implicitly: dropped nodes contribute cs=0."""
                for g in range(G):
                    if with_bias:
                        hm2 = HMF[:, 2 * g * N:2 * g * N + 2 * N]
                    else:
                        hmt = hmp.tile([128, 2 * N], bf16, tag="hm")
                        hm2 = hmt[:]
                    nc.vector.tensor_mul(hm2, HRAW[:, 2 * g * N:
                                                    2 * g * N + 2 * N],
                                         CSSB2[:, 2 * g * N:
                                               2 * g * N + 2 * N])
                    nc.vector.tensor_reduce(
                        RDTX[lvl][:, 2 * g:2 * g + 2],
                        hm2.rearrange("p (f n) -> p f n", f=2),
                        axis=AX.X, op=OP.max)
                    for ft in range(2):
                        rc = 2 * g + ft
                        sdum = hmp.tile([128, N], bf16, tag="sdum")
                        nc.scalar.activation(
                            sdum[:], hm2[:, ft * N:(ft + 1) * N], AF.Copy,
                            accum_out=RDTS[lvl][:, rc:rc + 1])

            # ---- the network
            # readout(L) is emitted inside level L+1's compute phase: its
            # products/reduces fill DVE/Scalar slack there instead of
            # piling onto the level-boundary serial chain.
            dense(0)
            fused_prop1()
            score(0)
            topk_pool(0)
            for lvl in range(1, 3):
                deg_c1()
                dense(lvl)
                bcast(lvl - 1)
                readout(lvl - 1)
                prop23()
                score(lvl)
                topk_pool(lvl)
            bcast(2)
            readout(2)

            # ---- combine readouts: z = sum_lvl [max | mean/k]
            def ftview(t, ft):
                # [128, 2G] (g, ft)-major -> [128, 1, G] slice for this ft
                return t[:].rearrange("p (g f) -> p f g", f=2)[:, ft:ft + 1, :]

            for kind in range(2):
                RD = RDTX if kind == 0 else RDTS
                for ft in range(2):
                    cg = (kind * 2 + ft) * G
                    dst = ZACC[:, cg:cg + G].rearrange("p (f g) -> p f g",
                                                       f=1)
                    v0, v1, v2 = (ftview(RD[l], ft) for l in range(3))
                    if kind == 0:
                        nc.vector.tensor_add(dst, v0, v1)
                        nc.vector.tensor_add(dst, dst, v2)
                    else:
                        nc.vector.tensor_scalar_mul(dst, v0, 1.0 / KS[0])
                        for l2, vv in ((1, v1), (2, v2)):
                            s3 = SCRC[:].rearrange("p (f g) -> p f g", f=1)
                            nc.vector.tensor_scalar_mul(s3, vv, 1.0 / KS[l2])
                            nc.vector.tensor_add(dst, dst, s3)

            # ---- final MLP + log_softmax
            ZB = sm.tile([128, 64], bf16, tag="zb")
            nc.vector.tensor_copy(ZB[:], ZACC[:])
            ps1 = pv.tile([16, H1], f32, tag="pcol")
            for kt in range(4):
                nc.tensor.matmul(ps1[0:16, :], ZB[:, kt * 16:(kt + 1) * 16],
                                 W1S[:, kt * H1:(kt + 1) * H1],
                                 start=(kt == 0), stop=False)
            nc.tensor.matmul(ps1[0:16, :], ONEB[0:1, 0:16], B1R[:],
                             start=False, stop=True)
            nc.scalar.activation(Z1[:], ps1[0:16, :], AF.Relu)
            for kt in range(2):
                pp = pt.tile([128, 128], f32, tag="pst")
                nc.tensor.transpose(pp[:, 0:16],
                                    Z1[0:16, kt * 128:(kt + 1) * 128],
                                    IDT[0:16, 0:16])
                nc.scalar.copy(Z1T[:, kt * G:(kt + 1) * G], pp[:, 0:16])
            ps2 = pv.tile([16, H2], f32, tag="pcol")
            for kt in range(2):
                nc.tensor.matmul(ps2[0:16, :], Z1T[:, kt * G:(kt + 1) * G],
                                 W2S[:, kt * H2:(kt + 1) * H2],
                                 start=(kt == 0), stop=False)
            nc.tensor.matmul(ps2[0:16, :], ONEB[0:1, 0:16], B2R[:],
                             start=False, stop=True)
            nc.scalar.activation(Z2[:], ps2[0:16, :], AF.Relu)
            pp = pt.tile([128, 128], f32, tag="pst")
            nc.tensor.transpose(pp[:, 0:16], Z2[0:16, :], IDT[0:16, 0:16])
            nc.scalar.copy(Z2T[:], pp[:, 0:16])
            ps3 = pv.tile([16, C], f32, tag="pcol")
            nc.tensor.matmul(ps3[0:16, :], Z2T[:], W3S[:], start=True,
                             stop=False)
            nc.tensor.matmul(ps3[0:16, :], ONEB[0:1, 0:16], B3R[:],
                             start=False, stop=True)
            nc.vector.tensor_reduce(M16[:], ps3[0:16, :], axis=AX.X, op=OP.max)
            nc.vector.tensor_scalar_mul(NM16[:], M16[:], -1.0)
            nc.scalar.activation(ES[:], ps3[0:16, :], AF.Exp,
                                 bias=NM16[0:16, 0:1], scale=1.0)
            nc.vector.tensor_reduce(SE[:], ES[:], axis=AX.X, op=OP.add)
            nc.scalar.activation(LSE[:], SE[:], AF.Ln)
            nc.vector.tensor_scalar(OUTS[:], ps3[0:16, :], M16[0:16, 0:1],
                                    LSE[0:16, 0:1], op0=OP.subtract,
                                    op1=OP.subtract)
            nc.sync.dma_start(out_d[:], OUTS[:])

    nc.compile()
    return nc


def _get_nc(with_bias, **kw):
    key = f"nc{int(with_bias)}{sorted(kw.items())}"
    if key not in _CACHE:
        _CACHE[key] = _build(with_bias, **kw)
    return _CACHE[key]


def _host_prep(inputs):
    import ml_dtypes
    bfd = ml_dtypes.bfloat16
    x = np.asarray(inputs["x"], np.float32)
    edges = np.asarray(inputs["edges"], np.int32)
    src = edges[..., 0].astype(np.int64)
    dst = edges[..., 1].astype(np.int64)
    gidx = np.arange(B, dtype=np.int64)[:, None]
    flat = (gidx * N * N + src * N + dst).ravel()
    A = np.bincount(flat, minlength=B * N * N).astype(np.float32)
    A = A.reshape(B, N, N)
    A += np.eye(N, dtype=np.float32)[None]

    # level-1 norms (keep-mask all ones) + fused 2-hop matrix
    degk = A.sum(axis=1)                       # [B, N]: col sums of W
    dinv1 = degk ** -0.5
    MADJ = np.matmul(A * (dinv1 ** 2)[:, None, :], A)  # (W D^2 W)[e, d]

    Ab = A.astype(bfd)
    Mb = MADJ.astype(bfd)
    xt = np.ascontiguousarray(
        x.reshape(NCORES, G * N, F_IN).transpose(0, 2, 1)).astype(bfd)

    # kd1 column tiles per core: [128, 2G], col = mt*G + g
    kd1 = np.zeros((NCORES, 128, 2 * G), np.float32)
    for c in range(NCORES):
        dv = dinv1[c * G:(c + 1) * G]          # [G, N]
        for mt in range(2):
            kd1[c, :, mt * G:(mt + 1) * G] = dv[:, mt * 128:(mt + 1) * 128].T

    esel = np.zeros((G, G * 128), np.float32)
    for g in range(G):
        esel[g, g * 128:(g + 1) * 128] = 1.0

    shared = {"esel": esel.astype(bfd)}
    for name, key in (("w12", "W12"), ("w22", "W22"), ("w32", "W32"),
                      ("w1", "W1"), ("w2", "W2"), ("w3", "W3")):
        shared[name] = np.ascontiguousarray(
            np.asarray(inputs[key], np.float32).astype(bfd))
    for name, key, n in (("b12", "b12", H1), ("b22", "b22", H1),
                         ("b32", "b32", H1), ("b1", "b1", H1),
                         ("b2", "b2", H2), ("b3", "b3", C)):
        shared[name] = np.asarray(inputs[key], np.float32).reshape(1, n) \
            .astype(bfd)
    pwc = np.zeros((128, 6), np.float32)
    for i, key in enumerate(("pw1", "pw2", "pw3")):
        pw = np.asarray(inputs[key], np.float32)
        pwn = pw / np.linalg.norm(pw)
        pwc[:, 2 * i] = pwn[:128]
        pwc[:, 2 * i + 1] = pwn[128:]
    shared["pwc"] = pwc.astype(bfd)

    with_bias = any(np.any(np.asarray(inputs[k])) for k in
                    ("b12", "b22", "b32"))
    in_maps = []
    for c in range(NCORES):
        m = dict(shared)
        m["xt"] = np.ascontiguousarray(xt[c])
        m["adj"] = np.ascontiguousarray(Ab[c * G:(c + 1) * G])
        m["madj"] = np.ascontiguousarray(Mb[c * G:(c + 1) * G])
        m["kd1"] = np.ascontiguousarray(kd1[c])
        in_maps.append(m)
    return in_maps, with_bias


def kernel(**inputs):
    from concourse.bass_utils import run_bass_kernel_spmd
    in_maps, with_bias = _host_prep(inputs)
    nc = _get_nc(with_bias)
    r = run_bass_kernel_spmd(nc, in_maps, core_ids=list(range(NCORES)))
    out = np.concatenate([r.results[c]["out"] for c in range(NCORES)], axis=0)
    return out.astype(np.float32)


def run_traced(inputs):
    """Like kernel() but with NTFF tracing; returns (out, BassKernelResults)."""
    import sys
    import types
    if "antenv.axon_hooks" not in sys.modules:
        hooks = types.ModuleType("antenv.axon_hooks")
        hooks._hook = None
        hooks.set_axon_ntff_profile_hook = lambda h: setattr(hooks, "_hook", h)
        hooks.get_axon_ntff_profile_hook = lambda: hooks._hook
        sys.modules["antenv.axon_hooks"] = hooks
        from trn_agent_boot.trn_boot import _ntff_profile_via_ctypes
        hooks.set_axon_ntff_profile_hook(
            _ntff_profile_via_ctypes("/opt/axon/libaxon_pjrt.so"))
    from concourse.bass_utils import run_bass_kernel_spmd
    in_maps, with_bias = _host_prep(inputs)
    nc = _get_nc(with_bias)
    r = run_bass_kernel_spmd(nc, in_maps, core_ids=list(range(NCORES)),
                             trace=True)
    out = np.concatenate([r.results[c]["out"] for c in range(NCORES)], axis=0)
    return out.astype(np.float32), r
